# revision 1
# baseline (speedup 1.0000x reference)
"""Trainium2 Bass kernel for nn_DenTargetTransformerConv (GNN message passing).

Strategy (graph/data parallel, dst-owner sharding across 8 NeuronCores):
  - Nodes are partitioned by dst-id range; each core owns N/8 nodes and all
    edges whose dst falls in its range. Cores are fully independent (the
    "halo exchange" of src features is materialized host-side as per-section
    compacted gather tables; the device gathers per-edge rows from them).
  - Per core, own nodes are sorted by in-degree and packed into groups of
    128 (SBUF partition dim). Every node in group g gets K[g] edge slots
    (K[g] = max degree in that group position across all cores, so the 8
    cores share one compiled program). Per-edge q||v rows (512B) are
    fetched with bulk dma_gather instructions; scores, edge softmax
    (max-subtraction elided: scores are O(+-5) in f32), and the weighted
    aggregation run on DVE/ACT with free-axis strided reduces. The small
    per-node linears run on PE with the bias folded in via a ones-row.
"""

import numpy as np

import concourse.bacc as bacc
import concourse.bass as bass
import concourse.tile as tile
from concourse import mybir
from concourse.bass_utils import run_bass_kernel_spmd

F32 = mybir.dt.float32
I16 = mybir.dt.int16
AX = mybir.AxisListType
ALU = mybir.AluOpType
ACTF = mybir.ActivationFunctionType

P = 128
NCORES = 8
HD = 64          # H * D
H, D = 4, 16
IN_F = 64

RUNC = 48        # max slot-columns per merged compute run


# ----------------------------------------------------------------- host prep

def _plan(q_src, v_src, feat, src, dst, ncores):
    n = feat.shape[0]
    npc = n // ncores
    ngrp = (npc + P - 1) // P
    grid = ngrp * P
    ndum = grid - npc

    qv = np.concatenate(
        [np.asarray(q_src, np.float32).reshape(n, HD),
         np.asarray(v_src, np.float32).reshape(n, HD)], axis=1)  # [n, 128]

    src = np.asarray(src).astype(np.int64)
    dst = np.asarray(dst).astype(np.int64)
    order = np.argsort(dst, kind="stable")
    dst_s, src_s = dst[order], src[order]
    bounds = np.searchsorted(dst_s, np.arange(ncores + 1) * npc)

    cores = []
    gmax = np.zeros((ncores, ngrp), np.int64)
    for c in range(ncores):
        lo, hi = bounds[c], bounds[c + 1]
        dstL = dst_s[lo:hi] - c * npc          # ascending
        srcL = src_s[lo:hi]
        deg = np.bincount(dstL, minlength=npc)
        starts = np.concatenate([[0], np.cumsum(deg)])
        rank = np.arange(len(dstL)) - starts[dstL]
        perm = np.argsort(deg, kind="stable")  # ascending degree
        pos_of = np.empty(npc, np.int64)
        pos_of[perm] = ndum + np.arange(npc)
        gd = np.zeros(grid, np.int64)
        gd[ndum:] = deg[perm]
        gmax[c] = gd.reshape(ngrp, P).max(1)
        cores.append(dict(dstL=dstL, srcL=srcL, rank=rank, perm=perm,
                          pos_of=pos_of))

    K = np.maximum(gmax.max(0), 1)             # shared per-group slot count
    colbase = np.concatenate([[0], np.cumsum(K)]).astype(np.int64)
    totc = int(colbase[-1])

    # Per-core halo-exchange buffer: each node's K[g] neighbor qv rows are
    # staged contiguously (rows shared by several owned nodes are replicated
    # per consumer), so the device gather needs one descriptor per node.
    # Row layout: colbase[g]*128 + p*K[g] + k  for slot (group g, node p, k).
    per_core = []
    for c in range(ncores):
        cd = cores[c]
        pos_e = cd["pos_of"][cd["dstL"]]       # grid position of each edge
        g_e = pos_e // P
        p_e = pos_e % P
        col_e = colbase[g_e] + cd["rank"]
        tab = np.zeros((totc * P, 2 * HD), np.float32)
        rows = colbase[g_e] * P + p_e * K[g_e] + cd["rank"]
        tab[rows] = qv[cd["srcL"]]
        mask_flat = np.zeros(totc * P, np.float32)
        mask_flat[col_e * P + p_e] = 1.0
        mask_w = mask_flat.reshape(totc, P).T.copy()               # [128, totc]
        per_core.append(dict(tab=tab, mask=mask_w))

    # featT with ones row, per core, grid-permuted: [IN_F+1, grid]
    featTs = []
    feat = np.asarray(feat, np.float32)
    for c in range(ncores):
        ft = np.zeros((IN_F + 1, grid), np.float32)
        ft[IN_F, :] = 1.0
        perm = cores[c]["perm"]
        ft[:IN_F, ndum:] = feat[c * npc + perm].T
        featTs.append(ft)

    # Merge consecutive equal-K groups into runs of <= RUNC slot-columns;
    # all run APs stay within the 3-free-dim ISA limit via (H,D)->HD and
    # (R,K)->RK collapses.
    runs = []
    g = 0
    while g < ngrp:
        k = int(K[g])
        ge = g + 1
        while ge < ngrp and int(K[ge]) == k and (ge - g + 1) * k <= RUNC:
            ge += 1
        runs.append((g, ge, k))
        g = ge
    mrc = max((g1 - g0) * k for (g0, g1, k) in runs)
    rmax = max(g1 - g0 for (g0, g1, k) in runs)

    # identity gather indices for the largest run, wrapped + replicated
    idx_flat = np.arange(P * rmax, dtype=np.int16)
    idx_w = np.tile(idx_flat.reshape(P * rmax // 16, 16).T, (8, 1))

    return dict(n=n, npc=npc, ngrp=ngrp, grid=grid, ndum=ndum, K=K,
                colbase=colbase, totc=totc, runs=runs, mrc=mrc, rmax=rmax,
                idx_w=idx_w, cores=cores, per_core=per_core, featTs=featTs)


# ------------------------------------------------------------- device build

def _build_nc(plan, ncores):
    ngrp, totc, runs = plan["ngrp"], plan["totc"], plan["runs"]
    grid = plan["grid"]
    mrc = plan["mrc"]
    colbase = plan["colbase"]

    nc = bacc.Bacc("TRN2", target_bir_lowering=False, debug=False,
                   num_devices=ncores)

    featT_d = nc.dram_tensor("featT", [IN_F + 1, grid], F32,
                             kind="ExternalInput").ap()
    rmax = plan["rmax"]
    idx_d = nc.dram_tensor("idx", [P, 8 * rmax], I16,
                           kind="ExternalInput").ap()
    mask_d = nc.dram_tensor("mask", [P, totc], F32, kind="ExternalInput").ap()
    tab_d = nc.dram_tensor("tab", [totc * P, 2 * HD], F32,
                           kind="ExternalInput").ap()
    wk_d = nc.dram_tensor("wk", [IN_F + 1, HD], F32, kind="ExternalInput").ap()
    wsk_d = nc.dram_tensor("wsk", [IN_F + 1, HD], F32, kind="ExternalInput").ap()
    # gate weights / ln params / prelu packed on one row:
    # [wg1' (64) | wg2' (64) | bgate (1) | gamma (64) | beta (64) | prelu_a (1)]
    par_d = nc.dram_tensor("par", [1, 258], F32, kind="ExternalInput").ap()
    out_d = nc.dram_tensor("out", [P, ngrp * HD], F32, kind="ExternalOutput").ap()

    with tile.TileContext(nc) as tc:
        with (
            tc.tile_pool(name="singles", bufs=1) as singles,
            tc.tile_pool(name="psum", bufs=4, space="PSUM") as psum,
            tc.tile_pool(name="qvp", bufs=3) as qvp,
            tc.tile_pool(name="scr", bufs=4) as scr,
            tc.tile_pool(name="exs", bufs=4) as exs,
        ):
            # ---- static loads
            featT = singles.tile([IN_F + 1, grid], F32)
            nc.sync.dma_start(out=featT[:], in_=featT_d[:])
            idx_sb = singles.tile([P, 8 * rmax], I16)
            nc.sync.dma_start(out=idx_sb[:], in_=idx_d[:])
            mask_sb = singles.tile([P, totc], F32)
            nc.sync.dma_start(out=mask_sb[:], in_=mask_d[:])
            wk_sb = singles.tile([IN_F + 1, HD], F32)
            nc.sync.dma_start(out=wk_sb[:], in_=wk_d[:])
            wsk_sb = singles.tile([IN_F + 1, HD], F32)
            nc.sync.dma_start(out=wsk_sb[:], in_=wsk_d[:])
            # broadcast params to all partitions (replicating DMA)
            parb = singles.tile([P, 258], F32)
            nc.gpsimd.dma_start(
                out=parb[:],
                in_=bass.AP(tensor=par_d.tensor, offset=par_d.offset,
                            ap=[[0, P], [1, 258]]))
            wg1 = parb[:, 0:64]
            wg2 = parb[:, 64:128]
            bg = parb[:, 128:129]
            gamma = parb[:, 129:193]
            beta = parb[:, 193:257]
            pa = parb[:, 257:258]

            # ---- per-node linears on PE: k = feat@Wk + bk, skip = feat@Wskip + bskip
            k_sb = singles.tile([P, ngrp * HD], F32)
            skip_sb = singles.tile([P, ngrp * HD], F32)
            for g in range(ngrp):
                lhsT = featT[:, g * P:(g + 1) * P]
                pk = psum.tile([P, HD], F32, tag="pk")
                nc.tensor.matmul(out=pk[:], lhsT=lhsT, rhs=wk_sb[:],
                                 start=True, stop=True)
                nc.scalar.activation(out=k_sb[:, g * HD:(g + 1) * HD],
                                     in_=pk[:], func=ACTF.Copy)
                ps = psum.tile([P, HD], F32, tag="pk")
                nc.tensor.matmul(out=ps[:], lhsT=lhsT, rhs=wsk_sb[:],
                                 start=True, stop=True)
                nc.scalar.activation(out=skip_sb[:, g * HD:(g + 1) * HD],
                                     in_=ps[:], func=ACTF.Copy)

            agg_sb = singles.tile([P, ngrp * HD], F32)
            den_sb = singles.tile([P, ngrp * H], F32)
            eps_t = singles.tile([P, 1], F32)
            nc.vector.memset(eps_t[:], 1e-5)

            # ---- edge phase: per merged run (R equal-K groups), gather the
            # nodes' contiguous neighbor blocks (one descriptor per node)
            # and reduce. All APs stay within 3 free dims.
            for (g0r, g1r, K) in runs:
                R = g1r - g0r
                RK = R * K
                r0 = int(colbase[g0r]) * P
                in_ap = tab_d[r0:r0 + RK * P, :].rearrange(
                    "(n k) e -> n (k e)", k=K)
                qv_t = qvp.tile([P, mrc * 2 * HD], F32, tag="qv")
                nc.gpsimd.dma_gather(
                    out_ap=qv_t[:, :RK * 2 * HD].rearrange(
                        "p (c e) -> p c e", e=K * 2 * HD),
                    in_ap=in_ap,
                    idxs_ap=idx_sb[:, :8 * R],
                    num_idxs=P * R,
                    num_idxs_reg=P * R,
                    elem_size=K * 2 * HD,
                    single_packet=False,
                )
                c0g = int(colbase[g0r])
                qv0 = qv_t[:, 0:1]
                pp = qv0.ap[0]

                # score: a[p, rk, h] = sum_d q[p,rk,h,d] * kdst[p,r,h,d]
                q3 = bass.AP(tensor=qv0.tensor, offset=qv0.offset,
                             ap=[pp, [2 * HD * K, R], [2 * HD, K], [1, HD]])
                kk = k_sb[:, g0r * HD:g1r * HD]
                kb = bass.AP(tensor=kk.tensor, offset=kk.offset,
                             ap=[kk.ap[0], [HD, R], [0, K], [1, HD]])
                prod = scr.tile([P, mrc * HD], F32, tag="scr")
                pv = prod[:, :RK * HD]
                p3 = bass.AP(tensor=pv.tensor, offset=pv.offset,
                             ap=[pv.ap[0], [K * HD, R], [HD, K], [1, HD]])
                nc.vector.tensor_tensor(out=p3, in0=q3, in1=kb, op=ALU.mult)

                ex = exs.tile([P, max(mrc, 2 * ngrp // H + 2) * H], F32,
                              tag="ex")
                e3 = bass.AP(tensor=ex[:].tensor, offset=ex[:].offset,
                             ap=[ex[:].ap[0], [H, RK], [1, H]])
                p3r = bass.AP(tensor=pv.tensor, offset=pv.offset,
                              ap=[pv.ap[0], [HD, RK], [D, H], [1, D]])
                nc.vector.tensor_reduce(out=e3, in_=p3r, axis=AX.X,
                                        op=ALU.add)
                # ex = exp(a/4) * mask
                exf = ex[:, :RK * H]
                nc.scalar.activation(out=exf, in_=exf, func=ACTF.Exp,
                                     scale=0.25)
                mm = mask_sb[:, c0g:c0g + RK]
                mb = bass.AP(tensor=mm.tensor, offset=mm.offset,
                             ap=[mm.ap[0], [1, RK], [0, H]])
                e2 = bass.AP(tensor=exf.tensor, offset=exf.offset,
                             ap=[exf.ap[0], [H, RK], [1, H]])
                nc.vector.tensor_tensor(out=e2, in0=e2, in1=mb, op=ALU.mult)
                # denom[p, r, h] = sum_k ex
                dd = den_sb[:, g0r * H:g1r * H]
                e4 = bass.AP(tensor=exf.tensor, offset=exf.offset,
                             ap=[exf.ap[0], [K * H, R], [1, H], [H, K]])
                nc.vector.tensor_reduce(out=dd, in_=e4, axis=AX.X,
                                        op=ALU.add)
                # w[p, rk, h, d] = v * ex ; agg[p, r, hd] = sum_k w
                v3 = bass.AP(tensor=qv0.tensor, offset=qv0.offset + HD,
                             ap=[pp, [2 * HD, RK], [D, H], [1, D]])
                eb = bass.AP(tensor=exf.tensor, offset=exf.offset,
                             ap=[exf.ap[0], [H, RK], [1, H], [0, D]])
                w_t = scr.tile([P, mrc * HD], F32, tag="scr")
                wv = w_t[:, :RK * HD]
                w3 = bass.AP(tensor=wv.tensor, offset=wv.offset,
                             ap=[wv.ap[0], [HD, RK], [D, H], [1, D]])
                nc.vector.tensor_tensor(out=w3, in0=v3, in1=eb, op=ALU.mult)
                aa = agg_sb[:, g0r * HD:g1r * HD]
                wr = bass.AP(tensor=wv.tensor, offset=wv.offset,
                             ap=[wv.ap[0], [K * HD, R], [1, HD], [HD, K]])
                nc.vector.tensor_reduce(out=aa, in_=wr, axis=AX.X,
                                        op=ALU.add)

            # ---- node phase
            NG = ngrp
            # dinv = 1 / (den + 1e-9)
            nc.vector.tensor_scalar(out=den_sb[:], in0=den_sb[:],
                                    scalar1=1e-9, scalar2=None, op0=ALU.add)
            nc.vector.reciprocal(out=den_sb[:], in_=den_sb[:])
            # rst = agg * dinv (broadcast over d)
            rst = agg_sb
            din = den_sb[:]
            dinb = bass.AP(tensor=din.tensor, offset=din.offset,
                           ap=[din.ap[0], [1, NG * H], [0, D]])
            r3 = rst[:].rearrange("p (c d) -> p c d", d=D)
            nc.vector.tensor_tensor(out=r3, in0=r3, in1=dinb, op=ALU.mult)

            # gate logits
            z_t = singles.tile([P, ngrp * HD], F32)
            gl = exs.tile([P, max(mrc, 2 * ngrp // H + 2) * H], F32, tag="ex")
            wg1b = bass.AP(tensor=wg1.tensor, offset=wg1.offset,
                           ap=[wg1.ap[0], [0, NG], [1, HD]])
            wg2b = bass.AP(tensor=wg2.tensor, offset=wg2.offset,
                           ap=[wg2.ap[0], [0, NG], [1, HD]])
            zv = z_t[:, :NG * HD].rearrange("p (c f) -> p c f", f=HD)
            nc.vector.tensor_tensor(out=zv, in0=skip_sb[:].rearrange(
                "p (c f) -> p c f", f=HD), in1=wg1b, op=ALU.mult)
            nc.vector.tensor_reduce(out=gl[:, 0:NG], in_=zv, axis=AX.X,
                                    op=ALU.add)
            nc.gpsimd.tensor_tensor(out=zv, in0=rst[:].rearrange(
                "p (c f) -> p c f", f=HD), in1=wg2b, op=ALU.mult)
            nc.vector.tensor_reduce(out=gl[:, NG:2 * NG], in_=zv, axis=AX.X,
                                    op=ALU.add)
            nc.vector.tensor_tensor(out=gl[:, 0:NG], in0=gl[:, 0:NG],
                                    in1=gl[:, NG:2 * NG], op=ALU.add)
            nc.scalar.activation(out=gl[:, 0:NG], in_=gl[:, 0:NG],
                                 func=ACTF.Sigmoid, bias=bg)
            # rst = rst + gate * (skip - rst)
            dif = z_t[:, :NG * HD]
            nc.vector.tensor_tensor(out=dif, in0=skip_sb[:], in1=rst[:],
                                    op=ALU.subtract)
            gv = gl[:, 0:NG]
            gb_ = bass.AP(tensor=gv.tensor, offset=gv.offset,
                          ap=[gv.ap[0], [1, NG], [0, HD]])
            d3v = dif.rearrange("p (c f) -> p c f", f=HD)
            nc.vector.tensor_tensor(out=d3v, in0=d3v, in1=gb_, op=ALU.mult)
            nc.gpsimd.tensor_tensor(out=rst[:], in0=rst[:], in1=dif,
                                    op=ALU.add)

            # LayerNorm
            mu = exs.tile([P, max(mrc, 2 * ngrp // H + 2) * H], F32, tag="ex")
            r3f = rst[:].rearrange("p (c f) -> p c f", f=HD)
            nc.vector.tensor_reduce(out=mu[:, 0:NG], in_=r3f, axis=AX.X,
                                    op=ALU.add)
            nc.vector.tensor_scalar(out=mu[:, 0:NG], in0=mu[:, 0:NG],
                                    scalar1=1.0 / HD, scalar2=None,
                                    op0=ALU.mult)
            mub = bass.AP(tensor=mu[:].tensor, offset=mu[:].offset,
                          ap=[mu[:].ap[0], [1, NG], [0, HD]])
            nc.vector.tensor_tensor(out=r3f, in0=r3f, in1=mub, op=ALU.subtract)
            sq = z_t[:, :NG * HD]
            nc.gpsimd.tensor_tensor(out=sq, in0=rst[:], in1=rst[:],
                                    op=ALU.mult)
            vs = mu[:, NG:2 * NG]
            nc.vector.tensor_reduce(out=vs, in_=sq.rearrange(
                "p (c f) -> p c f", f=HD), axis=AX.X, op=ALU.add)
            nc.scalar.activation(out=vs, in_=vs, func=ACTF.Sqrt,
                                 scale=1.0 / HD, bias=eps_t[:])
            nc.vector.reciprocal(out=vs, in_=vs)
            vsb = bass.AP(tensor=vs.tensor, offset=vs.offset,
                          ap=[vs.ap[0], [1, NG], [0, HD]])
            nc.vector.tensor_tensor(out=r3f, in0=r3f, in1=vsb, op=ALU.mult)
            gammab = bass.AP(tensor=gamma.tensor, offset=gamma.offset,
                             ap=[gamma.ap[0], [0, NG], [1, HD]])
            nc.vector.tensor_tensor(out=r3f, in0=r3f, in1=gammab, op=ALU.mult)
            betab = bass.AP(tensor=beta.tensor, offset=beta.offset,
                            ap=[beta.ap[0], [0, NG], [1, HD]])
            nc.gpsimd.tensor_tensor(out=r3f, in0=r3f, in1=betab, op=ALU.add)
            # prelu: max(x,0) + a*min(x,0)
            pos = z_t[:, :NG * HD]
            nc.vector.tensor_scalar(out=pos, in0=rst[:], scalar1=0.0,
                                    scalar2=None, op0=ALU.max)
            nc.vector.tensor_scalar(out=rst[:], in0=rst[:], scalar1=0.0,
                                    scalar2=None, op0=ALU.min)
            nc.vector.scalar_tensor_tensor(out=rst[:], in0=rst[:], scalar=pa,
                                           in1=pos, op0=ALU.mult, op1=ALU.add)
            nc.sync.dma_start(out=out_d[:], in_=rst[:])

    nc.compile()
    return nc


# ------------------------------------------------------------------- driver

_CACHE = {}


def _get_nc(plan, ncores):
    key = (tuple(plan["K"].tolist()), plan["grid"], plan["totc"], ncores)
    if key not in _CACHE:
        _CACHE[key] = _build_nc(plan, ncores)
    return _CACHE[key]


def _make_inmaps(plan, params, ncores):
    (Wk, bk, Wskip, bskip, Wgate, bgate, ln_gamma, ln_beta, prelu_a) = params
    wk = np.concatenate([np.asarray(Wk, np.float32),
                         np.asarray(bk, np.float32).reshape(1, HD)])
    wsk = np.concatenate([np.asarray(Wskip, np.float32),
                          np.asarray(bskip, np.float32).reshape(1, HD)])
    wg = np.asarray(Wgate, np.float32).reshape(3 * HD)
    par = np.zeros((1, 258), np.float32)
    par[0, 0:64] = wg[0:64] + wg[128:192]        # acts on skip
    par[0, 64:128] = wg[64:128] - wg[128:192]    # acts on rst
    par[0, 128] = np.float32(np.asarray(bgate).reshape(-1)[0])
    par[0, 129:193] = np.asarray(ln_gamma, np.float32)
    par[0, 193:257] = np.asarray(ln_beta, np.float32)
    par[0, 257] = np.float32(np.asarray(prelu_a).reshape(-1)[0])

    in_maps = []
    for c in range(ncores):
        pc = plan["per_core"][c]
        m = dict(featT=plan["featTs"][c], idx=plan["idx_w"], mask=pc["mask"],
                 tab=pc["tab"], wk=wk, wsk=wsk, par=par)
        in_maps.append(m)
    return in_maps


def run(q_src, v_src, feat, src, dst, Wk, bk, Wskip, bskip, Wgate, bgate,
        ln_gamma, ln_beta, prelu_a, ncores=NCORES, trace=False):
    plan = _plan(q_src, v_src, feat, src, dst, ncores)
    nc = _get_nc(plan, ncores)
    in_maps = _make_inmaps(
        plan, (Wk, bk, Wskip, bskip, Wgate, bgate, ln_gamma, ln_beta, prelu_a),
        ncores)
    res = run_bass_kernel_spmd(nc, in_maps, core_ids=list(range(ncores)),
                               trace=trace)
    n, npc, ngrp = plan["n"], plan["npc"], plan["ngrp"]
    out = np.empty((n, HD), np.float32)
    for c in range(ncores):
        r = res.results[c]["out"]                          # [128, ngrp*64]
        arr = r.reshape(P, ngrp, HD).transpose(1, 0, 2).reshape(-1, HD)
        out[c * npc + plan["cores"][c]["perm"]] = arr[plan["ndum"]:plan["ndum"] + npc]
    return out, res, plan, in_maps, nc


def kernel(**inputs):
    out, _, _, _, _ = run(**inputs)
    return out



# revision 5
# speedup vs baseline: 1.7533x; 1.7533x over previous
"""Trainium2 Bass kernel for nn_DenTargetTransformerConv (GNN message passing).

Strategy (graph/data parallel, dst-owner sharding across 8 NeuronCores):
  - Nodes are partitioned by dst-id range; each core owns N/8 nodes and all
    edges whose dst falls in its range. The "halo exchange" of src features is
    materialized host-side as per-core edge-slot tables (rows replicated per
    consumer), so the device reads are plain strided DMAs.
  - Per core, own nodes are sorted by in-degree and packed into groups of 128
    (SBUF partition dim). Consecutive groups are merged into equal-K runs
    (K = slots per node, shared across the 8 cores so one program serves all).
  - Edge phase per run: one static DMA brings the [128, RK*128] bf16 q||v
    slot tile; DVE computes per-slot q*k products and exp-weighted v in bf16
    (2x mode); the two big segment reductions (score over D, aggregation
    over K) run on the Tensor engine as identity-weight PSUM-accumulate
    matmul chains, with the softmax pad-mask folded in as one extra
    accumulated matmul of a -400 bias table. exp runs on the Scalar engine
    straight out of PSUM.
  - v (and everything downstream of the aggregation) lives in a (d,h)
    interleaved layout so the exp broadcast has a step-1 inner axis (DVE 2x
    mode); the host un-permutes the final output.
  - Node phase (softmax normalize, gate, LayerNorm, PReLU) runs in two
    group-halves so it overlaps the edge runs; per-node sums go to the
    Tensor engine, transcendentals to the Scalar engine.
"""

import numpy as np
import ml_dtypes

import concourse.bacc as bacc
import concourse.bass as bass
import concourse.tile as tile
from concourse import mybir
from concourse.bass_utils import run_bass_kernel_spmd

F32 = mybir.dt.float32
BF16 = mybir.dt.bfloat16
AX = mybir.AxisListType
ALU = mybir.AluOpType
ACTF = mybir.ActivationFunctionType
BF = ml_dtypes.bfloat16

P = 128
NCORES = 8
HD = 64          # H * D
H, D = 4, 16
IN_F = 64

RMAX = 8         # max groups per run (agg PSUM: R*64 <= 512)
RKMAX = 96       # max slot-columns per run (SBUF + score PSUM: RK*4 <= 512)
KSPREAD = 2      # max K padding when merging groups into a run

# natural hd = h*16+d  <->  stored j = d*4+h
_PERM = np.arange(HD).reshape(H, D).T.reshape(-1)       # j -> natural hd


# ----------------------------------------------------------------- host prep

def _plan(q_src, v_src, feat, src, dst, ncores):
    n = feat.shape[0]
    npc = n // ncores
    ngrp = (npc + P - 1) // P
    grid = ngrp * P
    ndum = grid - npc

    q2 = np.asarray(q_src, np.float32).reshape(n, HD)
    v2 = np.asarray(v_src, np.float32).reshape(n, H, D).transpose(0, 2, 1).reshape(n, HD)
    qv = np.concatenate([q2, v2], axis=1).astype(BF)    # [n, 128]

    src = np.asarray(src).astype(np.int64)
    dst = np.asarray(dst).astype(np.int64)
    order = np.argsort(dst, kind="stable")
    dst_s, src_s = dst[order], src[order]
    bounds = np.searchsorted(dst_s, np.arange(ncores + 1) * npc)

    cores = []
    gmax = np.zeros((ncores, ngrp), np.int64)
    for c in range(ncores):
        lo, hi = bounds[c], bounds[c + 1]
        dstL = dst_s[lo:hi] - c * npc          # ascending
        srcL = src_s[lo:hi]
        deg = np.bincount(dstL, minlength=npc)
        starts = np.concatenate([[0], np.cumsum(deg)])
        rank = np.arange(len(dstL)) - starts[dstL]
        perm = np.argsort(deg, kind="stable")  # ascending degree
        pos_of = np.empty(npc, np.int64)
        pos_of[perm] = ndum + np.arange(npc)
        gd = np.zeros(grid, np.int64)
        gd[ndum:] = deg[perm]
        gmax[c] = gd.reshape(ngrp, P).max(1)
        cores.append(dict(dstL=dstL, srcL=srcL, rank=rank, perm=perm,
                          pos_of=pos_of))

    K = np.maximum(gmax.max(0), 1)             # per-group slot count

    # merge consecutive groups into equal-K runs (pad K up to the run max)
    runs = []
    g = 0
    while g < ngrp:
        ge = g + 1
        while (ge < ngrp and ge - g < RMAX
               and (ge - g + 1) * K[ge] <= RKMAX
               and K[ge] - K[g] <= KSPREAD):
            ge += 1
        runs.append((g, ge, int(K[ge - 1])))
        g = ge
    rkbase = np.zeros(len(runs) + 1, np.int64)
    for i, (g0, g1, k) in enumerate(runs):
        rkbase[i + 1] = rkbase[i] + (g1 - g0) * k
    totrk = int(rkbase[-1])

    # per-core tables
    per_core = []
    grp_run = np.zeros(ngrp, np.int64)
    for i, (g0, g1, k) in enumerate(runs):
        grp_run[g0:g1] = i
    run_g0 = np.array([r[0] for r in runs])
    run_k = np.array([r[2] for r in runs])

    for c in range(ncores):
        cd = cores[c]
        pos_e = cd["pos_of"][cd["dstL"]]       # grid position of each edge
        g_e = pos_e // P
        p_e = pos_e % P
        i_e = grp_run[g_e]
        r_e = g_e - run_g0[i_e]
        k_e = run_k[i_e]
        # row = rkbase[i]*128 + p*(R*K) + r*K + rank  (partition-major)
        rk_run = np.array([r[1] - r[0] for r in runs])[i_e] * k_e
        rows = rkbase[i_e] * P + p_e * rk_run + r_e * k_e + cd["rank"]
        tab = np.zeros((totrk * P, 2 * HD), BF)
        tab[rows] = qv[cd["srcL"]]
        # maskneg: -400 on padded slots (exp -> 0), 0 on real slots
        mn = np.full((P, totrk), -400.0, np.float32)
        cols = rkbase[i_e] + r_e * k_e + cd["rank"]
        mn[p_e, cols] = 0.0
        maskneg = np.repeat(mn, H, axis=1).astype(BF)   # [128, totrk*4]
        per_core.append(dict(tab=tab, maskneg=maskneg))

    # featT with ones row, per core, grid-permuted: [IN_F+1, grid] bf16
    featTs = []
    feat = np.asarray(feat, np.float32)
    for c in range(ncores):
        ft = np.zeros((IN_F + 1, grid), np.float32)
        ft[IN_F, :] = 1.0
        perm = cores[c]["perm"]
        ft[:IN_F, ndum:] = feat[c * npc + perm].T
        featTs.append(ft.astype(BF))

    ident = np.eye(P, dtype=BF)

    return dict(n=n, npc=npc, ngrp=ngrp, grid=grid, ndum=ndum, K=K,
                runs=runs, rkbase=rkbase, totrk=totrk, ident=ident,
                cores=cores, per_core=per_core, featTs=featTs)


# ------------------------------------------------------------- device build

def _build_nc(plan, ncores):
    ngrp, runs, rkbase, totrk = (plan["ngrp"], plan["runs"], plan["rkbase"],
                                 plan["totrk"])
    grid = plan["grid"]
    G = ngrp

    nc = bacc.Bacc("TRN2", target_bir_lowering=False, debug=False,
                   num_devices=ncores)

    featT_d = nc.dram_tensor("featT", [IN_F + 1, grid], BF16,
                             kind="ExternalInput").ap()
    tab_d = nc.dram_tensor("tab", [totrk * P, 2 * HD], BF16,
                           kind="ExternalInput").ap()
    mask_d = nc.dram_tensor("maskneg", [P, totrk * H], BF16,
                            kind="ExternalInput").ap()
    ident_d = nc.dram_tensor("ident", [P, P], BF16, kind="ExternalInput").ap()
    wcat_d = nc.dram_tensor("wcat", [IN_F + 1, P], BF16,
                            kind="ExternalInput").ap()
    # bf16 params: [wg1' | wg2' | gamma' | beta'] (all (d,h)-permuted)
    parb_d = nc.dram_tensor("parb", [1, 4 * HD], BF16,
                            kind="ExternalInput").ap()
    # f32 params: [bgate, prelu_a, eps, pad]
    parf_d = nc.dram_tensor("parf", [1, 4], F32, kind="ExternalInput").ap()
    out_d = nc.dram_tensor("out", [P, G * HD], F32, kind="ExternalOutput").ap()

    NH = G // 2          # node-phase half boundary (in groups)
    halves = [(0, NH), (NH, G)]
    # first run index after which all groups < NH are aggregated
    half_done_run = min(i for i, (g0, g1, k) in enumerate(runs) if g1 >= NH)

    with tile.TileContext(nc) as tc:
        with (
            tc.tile_pool(name="singles", bufs=1) as singles,
            tc.tile_pool(name="plin", bufs=2, space="PSUM") as plin,
            tc.tile_pool(name="pscore", bufs=2, space="PSUM") as pscore,
            tc.tile_pool(name="pagg", bufs=2, space="PSUM") as pagg,
            tc.tile_pool(name="pnode", bufs=2, space="PSUM") as pnode,
            tc.tile_pool(name="qvp", bufs=2) as qvp,
            tc.tile_pool(name="prodp", bufs=2) as prodp,
            tc.tile_pool(name="wp", bufs=2) as wp,
            tc.tile_pool(name="exp_", bufs=2) as exp_,
            tc.tile_pool(name="nodep", bufs=2) as nodep,
            tc.tile_pool(name="smallp", bufs=2) as smallp,
        ):
            # ---- static loads
            featT = singles.tile([IN_F + 1, grid], BF16)
            nc.sync.dma_start(out=featT[:], in_=featT_d[:])
            wcat = singles.tile([IN_F + 1, P], BF16)
            nc.sync.dma_start(out=wcat[:], in_=wcat_d[:])
            ident = singles.tile([P, P], BF16)
            nc.sync.dma_start(out=ident[:], in_=ident_d[:])
            maskneg = singles.tile([P, totrk * H], BF16)
            nc.sync.dma_start(out=maskneg[:], in_=mask_d[:])
            parb = singles.tile([P, 4 * HD], BF16)
            nc.sync.dma_start(
                out=parb[:],
                in_=bass.AP(tensor=parb_d.tensor, offset=parb_d.offset,
                            ap=[[0, P], [1, 4 * HD]]))
            parf = singles.tile([P, 4], F32)
            nc.sync.dma_start(
                out=parf[:],
                in_=bass.AP(tensor=parf_d.tensor, offset=parf_d.offset,
                            ap=[[0, P], [1, 4]]))
            bg_ap = parf[:, 0:1]
            pa_ap = parf[:, 1:2]
            eps_ap = parf[:, 2:3]

            # persistent state
            ks_bf = singles.tile([P, G * P], BF16)     # per group: [k(64) | skip(64)]
            den = singles.tile([P, G * H], F32)
            agg_bf = singles.tile([P, G * HD], BF16)

            def pap(t, extra, off=0):
                sl = t[:, 0:1]
                return bass.AP(tensor=sl.tensor, offset=sl.offset + off,
                               ap=[sl.ap[0]] + extra)

            # ---- per-node linears: k|skip = featT_g.T @ wcat, 4 groups/bank
            for c0 in range(0, G, 4):
                cn = min(4, G - c0)
                pl = plin.tile([P, 4 * P], F32, tag="lin")
                for j in range(cn):
                    g = c0 + j
                    nc.tensor.matmul(out=pl[:, j * P:(j + 1) * P],
                                     lhsT=featT[:, g * P:(g + 1) * P],
                                     rhs=wcat[:], start=True, stop=True)
                nc.scalar.activation(out=ks_bf[:, c0 * P:(c0 + cn) * P],
                                     in_=pl[:, :cn * P], func=ACTF.Copy)

            # ---- edge phase
            for i, (g0, g1, K) in enumerate(runs):
                R = g1 - g0
                RK = R * K
                r0 = int(rkbase[i])

                qv = qvp.tile([P, RKMAX * 2 * HD], BF16, tag="qv")
                in_ap = tab_d[r0 * P:(r0 + RK) * P, :].rearrange(
                    "(p rk) e -> p (rk e)", p=P)
                nc.sync.dma_start(out=qv[:, :RK * 2 * HD], in_=in_ap)

                # prod[rk, h, d] = q[rk, h, d] * k_g[h, d]  (bf16 2x)
                prod = prodp.tile([P, RKMAX * HD], BF16, tag="prod")
                q3 = pap(qv, [[2 * HD * K, R], [2 * HD, K], [1, HD]])
                kb = pap(ks_bf, [[P, R], [0, K], [1, HD]], off=g0 * P)
                p3 = pap(prod, [[HD * K, R], [HD, K], [1, HD]])
                nc.vector.tensor_tensor(out=p3, in0=q3, in1=kb, op=ALU.mult)

                # score[rk, h] = sum_d prod  + maskneg   (PE accumulate)
                sp = pscore.tile([P, RKMAX * H], F32, tag="sp")
                for d in range(D):
                    rhs = pap(prod, [[HD, RK], [D, H]], off=d)
                    nc.tensor.matmul(out=sp[:, :RK * H], lhsT=ident[:],
                                     rhs=rhs, start=(d == 0), stop=False)
                nc.tensor.matmul(out=sp[:, :RK * H], lhsT=ident[:],
                                 rhs=maskneg[:, r0 * H:(r0 + RK) * H],
                                 start=False, stop=True)

                # ex = exp(score/4)  (bf16, straight out of PSUM)
                ex = exp_.tile([P, RKMAX * H], BF16, tag="ex")
                nc.scalar.activation(out=ex[:, :RK * H], in_=sp[:, :RK * H],
                                     func=ACTF.Exp, scale=0.25)

                # den[r, h] = sum_k ex
                e4 = pap(ex, [[K * H, R], [1, H], [H, K]])
                nc.vector.tensor_reduce(out=den[:, g0 * H:g1 * H], in_=e4,
                                        axis=AX.X, op=ALU.add)

                # w[rk, d, h] = v[rk, d, h] * ex[rk, h]  (bf16 2x)
                w = wp.tile([P, RKMAX * HD], BF16, tag="w")
                v3 = pap(qv, [[2 * HD, RK], [H, D], [1, H]], off=HD)
                eb = pap(ex, [[H, RK], [0, D], [1, H]])
                w3 = pap(w, [[HD, RK], [H, D], [1, H]])
                nc.vector.tensor_tensor(out=w3, in0=v3, in1=eb, op=ALU.mult)

                # agg[r, j] = sum_k w  (PE accumulate)
                ag = pagg.tile([P, RMAX * HD], F32, tag="agg")
                for k in range(K):
                    rhs = pap(w, [[K * HD, R], [1, HD]], off=k * HD)
                    nc.tensor.matmul(out=ag[:, :R * HD], lhsT=ident[:],
                                     rhs=rhs, start=(k == 0), stop=(k == K - 1))
                nc.scalar.activation(out=agg_bf[:, g0 * HD:g1 * HD],
                                     in_=ag[:, :R * HD], func=ACTF.Copy)

                # ---- node phase, interleaved per half
                todo = []
                if i == half_done_run:
                    todo.append(halves[0])
                if i == len(runs) - 1:
                    todo.append(halves[1])
                for h0, h1 in todo:
                    NG = h1 - h0
                    Fh = NG * HD

                    dv = smallp.tile([P, (G - NH) * H], F32, tag="dinv")
                    nc.vector.tensor_scalar(out=dv[:, :NG * H],
                                            in0=den[:, h0 * H:h1 * H],
                                            scalar1=1e-9, scalar2=None,
                                            op0=ALU.add)
                    nc.vector.reciprocal_approx_fast(out=dv[:, :NG * H],
                                                     in_=dv[:, :NG * H])
                    dvb = smallp.tile([P, (G - NH) * H], BF16, tag="dinvb")
                    nc.vector.tensor_copy(out=dvb[:, :NG * H],
                                          in_=dv[:, :NG * H])

                    # rst = agg * dinv
                    rst = nodep.tile([P, (G - NH) * HD], BF16, tag="rst")
                    dib = pap(dvb, [[H, NG], [0, D], [1, H]])
                    a3 = pap(agg_bf, [[HD, NG], [H, D], [1, H]], off=h0 * HD)
                    r3 = pap(rst, [[HD, NG], [H, D], [1, H]])
                    nc.vector.tensor_tensor(out=r3, in0=a3, in1=dib,
                                            op=ALU.mult)

                    # gate logits: z = [skip*wg1 | rst*wg2], sum over hd on PE
                    zc = nodep.tile([P, 2 * (G - NH) * HD], BF16, tag="zc")
                    sk = pap(ks_bf, [[P, NG], [1, HD]], off=h0 * P + HD)
                    wg1 = pap(parb, [[0, NG], [1, HD]], off=0)
                    wg2 = pap(parb, [[0, NG], [1, HD]], off=HD)
                    z1 = pap(zc, [[HD, NG], [1, HD]])
                    nc.vector.tensor_tensor(out=z1, in0=sk, in1=wg1,
                                            op=ALU.mult)
                    z2 = pap(zc, [[HD, NG], [1, HD]], off=Fh)
                    nc.vector.tensor_tensor(out=z2, in0=rst[:, :Fh], in1=wg2,
                                            op=ALU.mult)
                    pn = pnode.tile([P, 2 * (G - NH)], F32, tag="pn")
                    for hd in range(HD):
                        rhs = pap(zc, [[HD, 2 * NG]], off=hd)
                        nc.tensor.matmul(out=pn[:, :2 * NG], lhsT=ident[:],
                                         rhs=rhs, start=(hd == 0),
                                         stop=(hd == HD - 1))
                    lgs = smallp.tile([P, 2 * (G - NH)], F32, tag="lgs")
                    nc.scalar.activation(out=lgs[:, :2 * NG],
                                         in_=pn[:, :2 * NG], func=ACTF.Copy)
                    logit = smallp.tile([P, G - NH], F32, tag="logit")
                    nc.vector.tensor_tensor(out=logit[:, :NG],
                                            in0=lgs[:, :NG],
                                            in1=lgs[:, NG:2 * NG], op=ALU.add)
                    gate = smallp.tile([P, G - NH], BF16, tag="gate")
                    nc.scalar.activation(out=gate[:, :NG], in_=logit[:, :NG],
                                         func=ACTF.Sigmoid, bias=bg_ap)

                    # rst += gate * (skip - rst)
                    dif = nodep.tile([P, (G - NH) * HD], BF16, tag="dif")
                    nc.vector.tensor_tensor(out=dif[:, :Fh], in0=sk,
                                            in1=rst[:, :Fh], op=ALU.subtract)
                    gb = pap(gate, [[1, NG], [0, HD]])
                    d3 = pap(dif, [[HD, NG], [1, HD]])
                    nc.vector.tensor_tensor(out=d3, in0=d3, in1=gb,
                                            op=ALU.mult)
                    nc.vector.tensor_tensor(out=rst[:, :Fh], in0=rst[:, :Fh],
                                            in1=dif[:, :Fh], op=ALU.add)

                    # LayerNorm stats on PE: sum rst, sum rst^2
                    zc2 = nodep.tile([P, 2 * (G - NH) * HD], BF16, tag="zc")
                    nc.vector.tensor_copy(out=zc2[:, :Fh], in_=rst[:, :Fh])
                    nc.vector.tensor_tensor(out=zc2[:, Fh:2 * Fh],
                                            in0=rst[:, :Fh], in1=rst[:, :Fh],
                                            op=ALU.mult)
                    pn2 = pnode.tile([P, 2 * (G - NH)], F32, tag="pn")
                    for hd in range(HD):
                        rhs = pap(zc2, [[HD, 2 * NG]], off=hd)
                        nc.tensor.matmul(out=pn2[:, :2 * NG], lhsT=ident[:],
                                         rhs=rhs, start=(hd == 0),
                                         stop=(hd == HD - 1))
                    stats = smallp.tile([P, 2 * (G - NH)], F32, tag="stats")
                    nc.scalar.activation(out=stats[:, :2 * NG],
                                         in_=pn2[:, :2 * NG], func=ACTF.Copy,
                                         scale=1.0 / HD)
                    mu = stats[:, 0:NG]
                    msq = stats[:, NG:2 * NG]
                    var = smallp.tile([P, G - NH], F32, tag="var")
                    nc.vector.tensor_tensor(out=var[:, :NG], in0=mu, in1=mu,
                                            op=ALU.mult)
                    nc.vector.tensor_tensor(out=var[:, :NG], in0=msq,
                                            in1=var[:, :NG], op=ALU.subtract)
                    sd = smallp.tile([P, G - NH], F32, tag="sd")
                    nc.scalar.activation(out=sd[:, :NG], in_=var[:, :NG],
                                         func=ACTF.Sqrt, bias=eps_ap)
                    nc.vector.reciprocal_approx_fast(out=sd[:, :NG],
                                                     in_=sd[:, :NG])
                    mrs = smallp.tile([P, 2 * (G - NH)], BF16, tag="mrs")
                    nc.vector.tensor_copy(out=mrs[:, :NG], in_=mu)
                    nc.vector.tensor_copy(out=mrs[:, NG:2 * NG],
                                          in_=sd[:, :NG])

                    # xhat = (rst - mu) * rstd; out = prelu(xhat*gamma + beta)
                    mub = pap(mrs, [[1, NG], [0, HD]])
                    nc.vector.tensor_tensor(out=rst[:, :Fh], in0=rst[:, :Fh],
                                            in1=mub, op=ALU.subtract)
                    rsb = pap(mrs, [[1, NG], [0, HD]], off=NG)
                    nc.vector.tensor_tensor(out=rst[:, :Fh], in0=rst[:, :Fh],
                                            in1=rsb, op=ALU.mult)
                    gmb = pap(parb, [[0, NG], [1, HD]], off=2 * HD)
                    nc.vector.tensor_tensor(out=rst[:, :Fh], in0=rst[:, :Fh],
                                            in1=gmb, op=ALU.mult)
                    btb = pap(parb, [[0, NG], [1, HD]], off=3 * HD)
                    nc.gpsimd.tensor_tensor(out=rst[:, :Fh], in0=rst[:, :Fh],
                                            in1=btb, op=ALU.add)
                    outf = nodep.tile([P, (G - NH) * HD], F32, tag="outf")
                    nc.scalar.activation(out=outf[:, :Fh], in_=rst[:, :Fh],
                                         func=ACTF.Prelu, alpha=pa_ap)
                    nc.sync.dma_start(out=out_d[:, h0 * HD:h1 * HD],
                                      in_=outf[:, :Fh])

    nc.compile()
    return nc


# ------------------------------------------------------------------- driver

_CACHE = {}


def _get_nc(plan, ncores):
    key = (tuple(int(k) for g0, g1, k in plan["runs"]),
           tuple(g1 - g0 for g0, g1, k in plan["runs"]),
           plan["grid"], ncores)
    if key not in _CACHE:
        _CACHE[key] = _build_nc(plan, ncores)
    return _CACHE[key]


def _make_inmaps(plan, params, ncores):
    (Wk, bk, Wskip, bskip, Wgate, bgate, ln_gamma, ln_beta, prelu_a) = params
    Wk = np.asarray(Wk, np.float32)
    bk = np.asarray(bk, np.float32)
    Wsp = np.asarray(Wskip, np.float32)[:, _PERM]
    bsp = np.asarray(bskip, np.float32)[_PERM]
    wcat = np.zeros((IN_F + 1, P), np.float32)
    wcat[:IN_F, :HD] = Wk
    wcat[IN_F, :HD] = bk
    wcat[:IN_F, HD:] = Wsp
    wcat[IN_F, HD:] = bsp
    wcat = wcat.astype(BF)

    wg = np.asarray(Wgate, np.float32).reshape(3 * HD)
    parb = np.zeros((1, 4 * HD), np.float32)
    parb[0, 0:HD] = (wg[0:HD] + wg[2 * HD:3 * HD])[_PERM]       # on skip
    parb[0, HD:2 * HD] = (wg[HD:2 * HD] - wg[2 * HD:3 * HD])[_PERM]  # on rst
    parb[0, 2 * HD:3 * HD] = np.asarray(ln_gamma, np.float32)[_PERM]
    parb[0, 3 * HD:4 * HD] = np.asarray(ln_beta, np.float32)[_PERM]
    parb = parb.astype(BF)
    parf = np.zeros((1, 4), np.float32)
    parf[0, 0] = np.float32(np.asarray(bgate).reshape(-1)[0])
    parf[0, 1] = np.float32(np.asarray(prelu_a).reshape(-1)[0])
    parf[0, 2] = 1e-5

    in_maps = []
    for c in range(ncores):
        pc = plan["per_core"][c]
        m = dict(featT=plan["featTs"][c], tab=pc["tab"],
                 maskneg=pc["maskneg"], ident=plan["ident"],
                 wcat=wcat, parb=parb, parf=parf)
        in_maps.append(m)
    return in_maps


def run(q_src, v_src, feat, src, dst, Wk, bk, Wskip, bskip, Wgate, bgate,
        ln_gamma, ln_beta, prelu_a, ncores=NCORES, trace=False):
    plan = _plan(q_src, v_src, feat, src, dst, ncores)
    nc = _get_nc(plan, ncores)
    in_maps = _make_inmaps(
        plan, (Wk, bk, Wskip, bskip, Wgate, bgate, ln_gamma, ln_beta, prelu_a),
        ncores)
    res = run_bass_kernel_spmd(nc, in_maps, core_ids=list(range(ncores)),
                               trace=trace)
    n, npc, ngrp = plan["n"], plan["npc"], plan["ngrp"]
    out = np.empty((n, HD), np.float32)
    for c in range(ncores):
        r = np.asarray(res.results[c]["out"])              # [128, ngrp*64]
        r = r.reshape(P, ngrp, D, H).transpose(1, 0, 3, 2)  # -> [g, p, h, d]
        arr = r.reshape(-1, HD)
        out[c * npc + plan["cores"][c]["perm"]] = \
            arr[plan["ndum"]:plan["ndum"] + npc]
    return out, res, plan, in_maps, nc


def kernel(**inputs):
    out, _, _, _, _ = run(**inputs)
    return out


# revision 11
# speedup vs baseline: 2.1644x; 1.2345x over previous
"""Trainium2 Bass kernel for nn_DenTargetTransformerConv (GNN message passing).

Strategy (graph/data parallel, dst-owner sharding across 8 NeuronCores):
  - Nodes are partitioned by dst-id range; each core owns N/8 nodes and all
    edges whose dst falls in its range. The "halo exchange" of src features is
    materialized host-side as per-core edge-slot tables (rows replicated per
    consumer), so the device reads are plain strided DMAs.
  - Per core, own nodes are sorted by in-degree and packed into groups of 128
    (SBUF partition dim). Consecutive groups are merged into equal-K runs
    (K = slots per node, shared across the 8 cores so one program serves all).
  - Edge phase per run: one static DMA brings the [128, RK*128] bf16 q||v
    slot tile; DVE computes per-slot q*k products and exp-weighted v in bf16
    (2x mode); the two big segment reductions (score over D, aggregation
    over K) run on the Tensor engine as identity-weight PSUM-accumulate
    matmul chains, with the softmax pad-mask folded in as one extra
    accumulated matmul of a -400 bias table. exp runs on the Scalar engine
    straight out of PSUM.
  - v (and everything downstream of the aggregation) lives in a (d,h)
    interleaved layout so the exp broadcast has a step-1 inner axis (DVE 2x
    mode); the host un-permutes the final output.
  - Node phase (softmax normalize, gate, LayerNorm, PReLU) runs in two
    group-halves so it overlaps the edge runs; per-node sums go to the
    Tensor engine, transcendentals to the Scalar engine.
"""

import numpy as np
import ml_dtypes

import concourse.bacc as bacc
import concourse.bass as bass
import concourse.tile as tile
from concourse import mybir
from concourse.bass_utils import run_bass_kernel_spmd

F32 = mybir.dt.float32
BF16 = mybir.dt.bfloat16
AX = mybir.AxisListType
ALU = mybir.AluOpType
ACTF = mybir.ActivationFunctionType
BF = ml_dtypes.bfloat16

P = 128
NCORES = 8
HD = 64          # H * D
H, D = 4, 16
IN_F = 64

RMAX = 8         # max groups per run (agg PSUM: R*64 <= 512)
RKMAX = 88       # max slot-columns per run (SBUF + score PSUM: RK*4 <= 512)
KSPREAD = 2      # max K padding when merging groups into a run

# natural hd = h*16+d  <->  stored j = d*4+h
_PERM = np.arange(HD).reshape(H, D).T.reshape(-1)       # j -> natural hd


# ----------------------------------------------------------------- host prep

def _plan(q_src, v_src, feat, src, dst, ncores):
    n = feat.shape[0]
    npc = n // ncores
    ngrp = (npc + P - 1) // P
    grid = ngrp * P
    ndum = grid - npc

    q2 = np.asarray(q_src, np.float32).reshape(n, HD)
    v2 = np.asarray(v_src, np.float32).reshape(n, H, D).transpose(0, 2, 1).reshape(n, HD)
    qv = np.concatenate([q2, v2], axis=1).astype(BF)    # [n, 128]

    src = np.asarray(src).astype(np.int64)
    dst = np.asarray(dst).astype(np.int64)
    order = np.argsort(dst, kind="stable")
    dst_s, src_s = dst[order], src[order]
    bounds = np.searchsorted(dst_s, np.arange(ncores + 1) * npc)

    cores = []
    gmax = np.zeros((ncores, ngrp), np.int64)
    for c in range(ncores):
        lo, hi = bounds[c], bounds[c + 1]
        dstL = dst_s[lo:hi] - c * npc          # ascending
        srcL = src_s[lo:hi]
        deg = np.bincount(dstL, minlength=npc)
        starts = np.concatenate([[0], np.cumsum(deg)])
        rank = np.arange(len(dstL)) - starts[dstL]
        perm = np.argsort(deg, kind="stable")  # ascending degree
        pos_of = np.empty(npc, np.int64)
        pos_of[perm] = ndum + np.arange(npc)
        gd = np.zeros(grid, np.int64)
        gd[ndum:] = deg[perm]
        gmax[c] = gd.reshape(ngrp, P).max(1)
        cores.append(dict(dstL=dstL, srcL=srcL, rank=rank, perm=perm,
                          pos_of=pos_of))

    K = np.maximum(gmax.max(0), 1)             # per-group slot count

    # merge consecutive groups into equal-K runs (pad K up to the run max)
    runs = []
    g = 0
    while g < ngrp:
        ge = g + 1
        while (ge < ngrp and ge - g < RMAX
               and (ge - g + 1) * K[ge] <= RKMAX
               and K[ge] - K[g] <= KSPREAD):
            ge += 1
        runs.append((g, ge, int(K[ge - 1])))
        g = ge
    rkbase = np.zeros(len(runs) + 1, np.int64)
    for i, (g0, g1, k) in enumerate(runs):
        rkbase[i + 1] = rkbase[i] + (g1 - g0) * k
    totrk = int(rkbase[-1])

    # per-core tables
    per_core = []
    grp_run = np.zeros(ngrp, np.int64)
    for i, (g0, g1, k) in enumerate(runs):
        grp_run[g0:g1] = i
    run_g0 = np.array([r[0] for r in runs])
    run_k = np.array([r[2] for r in runs])

    for c in range(ncores):
        cd = cores[c]
        pos_e = cd["pos_of"][cd["dstL"]]       # grid position of each edge
        g_e = pos_e // P
        p_e = pos_e % P
        i_e = grp_run[g_e]
        r_e = g_e - run_g0[i_e]
        k_e = run_k[i_e]
        # row = rkbase[i]*128 + p*(R*K) + r*K + rank  (partition-major)
        rk_run = np.array([r[1] - r[0] for r in runs])[i_e] * k_e
        rows = rkbase[i_e] * P + p_e * rk_run + r_e * k_e + cd["rank"]
        tab = np.zeros((totrk * P, 2 * HD), BF)
        tab[rows] = qv[cd["srcL"]]
        # maskneg: -400 on padded slots (exp -> 0), 0 on real slots
        mn = np.full((P, totrk), -400.0, np.float32)
        cols = rkbase[i_e] + r_e * k_e + cd["rank"]
        mn[p_e, cols] = 0.0
        maskneg = np.repeat(mn, H, axis=1).astype(BF)   # [128, totrk*4]
        per_core.append(dict(tab=tab, maskneg=maskneg))

    # featT with ones row, per core, grid-permuted: [IN_F+1, grid] bf16
    featTs = []
    feat = np.asarray(feat, np.float32)
    for c in range(ncores):
        ft = np.zeros((IN_F + 1, grid), np.float32)
        ft[IN_F, :] = 1.0
        perm = cores[c]["perm"]
        ft[:IN_F, ndum:] = feat[c * npc + perm].T
        featTs.append(ft.astype(BF))

    ident = np.eye(P, dtype=BF)

    return dict(n=n, npc=npc, ngrp=ngrp, grid=grid, ndum=ndum, K=K,
                runs=runs, rkbase=rkbase, totrk=totrk, ident=ident,
                cores=cores, per_core=per_core, featTs=featTs)


# ------------------------------------------------------------- device build

def _build_nc(plan, ncores):
    ngrp, runs, rkbase, totrk = (plan["ngrp"], plan["runs"], plan["rkbase"],
                                 plan["totrk"])
    grid = plan["grid"]
    G = ngrp

    nc = bacc.Bacc("TRN2", target_bir_lowering=False, debug=False,
                   num_devices=ncores)

    featT_d = nc.dram_tensor("featT", [IN_F + 1, grid], BF16,
                             kind="ExternalInput").ap()
    tab_d = nc.dram_tensor("tab", [totrk * P, 2 * HD], BF16,
                           kind="ExternalInput").ap()
    mask_d = nc.dram_tensor("maskneg", [P, totrk * H], BF16,
                            kind="ExternalInput").ap()
    ident_d = nc.dram_tensor("ident", [P, P], BF16, kind="ExternalInput").ap()
    wcat_d = nc.dram_tensor("wcat", [IN_F + 1, P], BF16,
                            kind="ExternalInput").ap()
    # bf16 params: [wg1' | wg2' | gamma' | beta'] (all (d,h)-permuted)
    parb_d = nc.dram_tensor("parb", [1, 4 * HD], BF16,
                            kind="ExternalInput").ap()
    # f32 params: [bgate, prelu_a, eps, pad]
    parf_d = nc.dram_tensor("parf", [1, 4], F32, kind="ExternalInput").ap()
    out_d = nc.dram_tensor("out", [P, G * HD], F32, kind="ExternalOutput").ap()

    NH = G // 2          # node-phase half boundary (in groups)
    halves = [(0, NH), (NH, G)]
    # first run index after which all groups < NH are aggregated
    half_done_run = min(i for i, (g0, g1, k) in enumerate(runs) if g1 >= NH)

    with tile.TileContext(nc) as tc:
        with (
            tc.tile_pool(name="singles", bufs=1) as singles,
            tc.tile_pool(name="plin", bufs=2, space="PSUM") as plin,
            tc.tile_pool(name="pscore", bufs=2, space="PSUM") as pscore,
            tc.tile_pool(name="pagg", bufs=2, space="PSUM") as pagg,
            tc.tile_pool(name="qvp", bufs=2) as qvp,
            tc.tile_pool(name="prodp", bufs=2) as prodp,
            tc.tile_pool(name="halfp", bufs=2) as halfp,
            tc.tile_pool(name="quartp", bufs=2) as quartp,
            tc.tile_pool(name="wp", bufs=2) as wp,
            tc.tile_pool(name="whp", bufs=2) as whp,
            tc.tile_pool(name="exp_", bufs=2) as exp_,
            tc.tile_pool(name="nodep", bufs=2) as nodep,
            tc.tile_pool(name="smallp", bufs=2) as smallp,
        ):
            # ---- static loads
            featT = singles.tile([IN_F + 1, grid], BF16)
            nc.sync.dma_start(out=featT[:], in_=featT_d[:])
            wcat = singles.tile([IN_F + 1, P], BF16)
            nc.sync.dma_start(out=wcat[:], in_=wcat_d[:])
            ident = singles.tile([P, P], BF16)
            nc.sync.dma_start(out=ident[:], in_=ident_d[:])
            maskneg = singles.tile([P, totrk * H], BF16)
            nc.sync.dma_start(out=maskneg[:], in_=mask_d[:])
            parb = singles.tile([P, 4 * HD], BF16)
            nc.sync.dma_start(
                out=parb[:],
                in_=bass.AP(tensor=parb_d.tensor, offset=parb_d.offset,
                            ap=[[0, P], [1, 4 * HD]]))
            parf = singles.tile([P, 4], F32)
            nc.sync.dma_start(
                out=parf[:],
                in_=bass.AP(tensor=parf_d.tensor, offset=parf_d.offset,
                            ap=[[0, P], [1, 4]]))
            bg_ap = parf[:, 0:1]
            pa_ap = parf[:, 1:2]
            eps_ap = parf[:, 2:3]

            # persistent state
            ks_bf = singles.tile([P, G * P], BF16)     # per group: [k(64) | skip(64)]
            den = singles.tile([P, G * H], F32)
            agg_bf = singles.tile([P, G * HD], BF16)

            def pap(t, extra, off=0):
                sl = t[:, 0:1]
                return bass.AP(tensor=sl.tensor, offset=sl.offset + off,
                               ap=[sl.ap[0]] + extra)

            # ---- per-node linears: k|skip = featT_g.T @ wcat, 4 groups/bank
            for c0 in range(0, G, 4):
                cn = min(4, G - c0)
                pl = plin.tile([P, 4 * P], F32, tag="lin")
                for j in range(cn):
                    g = c0 + j
                    nc.tensor.matmul(out=pl[:, j * P:(j + 1) * P],
                                     lhsT=featT[:, g * P:(g + 1) * P],
                                     rhs=wcat[:], start=True, stop=True)
                nc.scalar.activation(out=ks_bf[:, c0 * P:(c0 + cn) * P],
                                     in_=pl[:, :cn * P], func=ACTF.Copy)

            # ---- edge phase
            for i, (g0, g1, K) in enumerate(runs):
                R = g1 - g0
                RK = R * K
                r0 = int(rkbase[i])

                qv = qvp.tile([P, RKMAX * 2 * HD], BF16, tag="qv")
                in_ap = tab_d[r0 * P:(r0 + RK) * P, :].rearrange(
                    "(p rk) e -> p (rk e)", p=P)
                nc.sync.dma_start(out=qv[:, :RK * 2 * HD], in_=in_ap)

                # prod[rk, h, d] = q[rk, h, d] * k_g[h, d]  (bf16 2x)
                prod = prodp.tile([P, RKMAX * HD], BF16, tag="prod")
                q3 = pap(qv, [[2 * HD * K, R], [2 * HD, K], [1, HD]])
                kb = pap(ks_bf, [[P, R], [0, K], [1, HD]], off=g0 * P)
                p3 = pap(prod, [[HD * K, R], [HD, K], [1, HD]])
                nc.vector.tensor_tensor(out=p3, in0=q3, in1=kb, op=ALU.mult)

                # two pairwise pre-add stages on DVE: 16 d-slices -> 4
                ph = halfp.tile([P, RKMAX * 32], BF16, tag="ph")
                nc.vector.tensor_tensor(
                    out=pap(ph, [[32, RK], [8, H], [1, 8]]),
                    in0=pap(prod, [[HD, RK], [D, H], [1, 8]]),
                    in1=pap(prod, [[HD, RK], [D, H], [1, 8]], off=8),
                    op=ALU.add)
                pq = quartp.tile([P, RKMAX * 16], BF16, tag="pq")
                nc.vector.tensor_tensor(
                    out=pap(pq, [[16, RK], [4, H], [1, 4]]),
                    in0=pap(ph, [[32, RK], [8, H], [1, 4]]),
                    in1=pap(ph, [[32, RK], [8, H], [1, 4]], off=4),
                    op=ALU.add)

                # score[rk, h] = sum_d4 pq  + maskneg   (PE accumulate)
                sp = pscore.tile([P, RKMAX * H], F32, tag="sp")
                for d in range(4):
                    rhs = pap(pq, [[16, RK], [4, H]], off=d)
                    nc.tensor.matmul(out=sp[:, :RK * H], lhsT=ident[:],
                                     rhs=rhs, start=(d == 0), stop=False)
                nc.tensor.matmul(out=sp[:, :RK * H], lhsT=ident[:],
                                 rhs=maskneg[:, r0 * H:(r0 + RK) * H],
                                 start=False, stop=True)

                # ex = exp(score/4)  (bf16, straight out of PSUM)
                ex = exp_.tile([P, RKMAX * H], BF16, tag="ex")
                nc.scalar.activation(out=ex[:, :RK * H], in_=sp[:, :RK * H],
                                     func=ACTF.Exp, scale=0.25)

                # den[r, h] = sum_k ex
                e4 = pap(ex, [[K * H, R], [1, H], [H, K]])
                nc.vector.tensor_reduce(out=den[:, g0 * H:g1 * H], in_=e4,
                                        axis=AX.X, op=ALU.add)

                # w[rk, d, h] = v[rk, d, h] * ex[rk, h]  (bf16 2x)
                w = wp.tile([P, RKMAX * HD], BF16, tag="w")
                v3 = pap(qv, [[2 * HD, RK], [H, D], [1, H]], off=HD)
                eb = pap(ex, [[H, RK], [0, D], [1, H]])
                w3 = pap(w, [[HD, RK], [H, D], [1, H]])
                nc.vector.tensor_tensor(out=w3, in0=v3, in1=eb, op=ALU.mult)

                # pairwise k pre-add on DVE: K slices -> ceil(K/2)
                KH = K // 2
                wh = whp.tile([P, RKMAX * 32], BF16, tag="wh")
                if KH > 0:
                    nc.vector.tensor_tensor(
                        out=pap(wh, [[KH * HD, R], [1, KH * HD]]),
                        in0=pap(w, [[K * HD, R], [2 * HD, KH], [1, HD]]),
                        in1=pap(w, [[K * HD, R], [2 * HD, KH], [1, HD]],
                                off=HD),
                        op=ALU.add)

                # agg[r, j] = sum_k' wh (+ odd leftover)  (PE accumulate)
                ag = pagg.tile([P, RMAX * HD], F32, tag="agg")
                nmm = KH + (K % 2)
                for k in range(KH):
                    rhs = pap(wh, [[KH * HD, R], [1, HD]], off=k * HD)
                    nc.tensor.matmul(out=ag[:, :R * HD], lhsT=ident[:],
                                     rhs=rhs, start=(k == 0),
                                     stop=(k == nmm - 1))
                if K % 2:
                    rhs = pap(w, [[K * HD, R], [1, HD]], off=(K - 1) * HD)
                    nc.tensor.matmul(out=ag[:, :R * HD], lhsT=ident[:],
                                     rhs=rhs, start=(KH == 0), stop=True)
                nc.scalar.activation(out=agg_bf[:, g0 * HD:g1 * HD],
                                     in_=ag[:, :R * HD], func=ACTF.Copy)

                # ---- node phase, interleaved per half
                todo = []
                if i == half_done_run:
                    todo.append(halves[0])
                if i == len(runs) - 1:
                    todo.append(halves[1])
                for h0, h1 in todo:
                    NG = h1 - h0
                    Fh = NG * HD

                    dv = smallp.tile([P, (G - NH) * H], F32, tag="dinv")
                    nc.vector.tensor_scalar(out=dv[:, :NG * H],
                                            in0=den[:, h0 * H:h1 * H],
                                            scalar1=1e-9, scalar2=None,
                                            op0=ALU.add)
                    nc.vector.reciprocal_approx_fast(out=dv[:, :NG * H],
                                                     in_=dv[:, :NG * H])
                    dvb = smallp.tile([P, (G - NH) * H], BF16, tag="dinvb")
                    nc.vector.tensor_copy(out=dvb[:, :NG * H],
                                          in_=dv[:, :NG * H])

                    # rst = agg * dinv
                    rst = nodep.tile([P, (G - NH) * HD], BF16, tag="rst")
                    dib = pap(dvb, [[H, NG], [0, D], [1, H]])
                    a3 = pap(agg_bf, [[HD, NG], [H, D], [1, H]], off=h0 * HD)
                    r3 = pap(rst, [[HD, NG], [H, D], [1, H]])
                    nc.vector.tensor_tensor(out=r3, in0=a3, in1=dib,
                                            op=ALU.mult)

                    # gate logits: z = [skip*wg1 | rst*wg2], sum over hd (DVE)
                    zc = nodep.tile([P, 2 * (G - NH) * HD], BF16, tag="zc")
                    sk = pap(ks_bf, [[P, NG], [1, HD]], off=h0 * P + HD)
                    wg1 = pap(parb, [[0, NG], [1, HD]], off=0)
                    wg2 = pap(parb, [[0, NG], [1, HD]], off=HD)
                    z1 = pap(zc, [[HD, NG], [1, HD]])
                    nc.vector.tensor_tensor(out=z1, in0=sk, in1=wg1,
                                            op=ALU.mult)
                    z2 = pap(zc, [[HD, NG], [1, HD]], off=Fh)
                    nc.vector.tensor_tensor(out=z2, in0=rst[:, :Fh], in1=wg2,
                                            op=ALU.mult)
                    lgs = smallp.tile([P, 2 * (G - NH)], F32, tag="lgs")
                    nc.vector.tensor_reduce(
                        out=lgs[:, :2 * NG],
                        in_=pap(zc, [[HD, 2 * NG], [1, HD]]),
                        axis=AX.X, op=ALU.add)
                    logit = smallp.tile([P, G - NH], F32, tag="logit")
                    nc.vector.tensor_tensor(out=logit[:, :NG],
                                            in0=lgs[:, :NG],
                                            in1=lgs[:, NG:2 * NG], op=ALU.add)
                    gate = smallp.tile([P, G - NH], BF16, tag="gate")
                    nc.scalar.activation(out=gate[:, :NG], in_=logit[:, :NG],
                                         func=ACTF.Sigmoid, bias=bg_ap)

                    # rst += gate * (skip - rst)
                    dif = nodep.tile([P, (G - NH) * HD], BF16, tag="dif")
                    nc.vector.tensor_tensor(out=dif[:, :Fh], in0=sk,
                                            in1=rst[:, :Fh], op=ALU.subtract)
                    gb = pap(gate, [[1, NG], [0, HD]])
                    d3 = pap(dif, [[HD, NG], [1, HD]])
                    nc.gpsimd.tensor_tensor(out=d3, in0=d3, in1=gb,
                                            op=ALU.mult)
                    nc.vector.tensor_tensor(out=rst[:, :Fh], in0=rst[:, :Fh],
                                            in1=dif[:, :Fh], op=ALU.add)

                    # LayerNorm stats: sum rst, sum rst^2 (DVE)
                    zc2 = nodep.tile([P, 2 * (G - NH) * HD], BF16, tag="zc")
                    nc.vector.tensor_copy(out=zc2[:, :Fh], in_=rst[:, :Fh])
                    nc.vector.tensor_tensor(out=zc2[:, Fh:2 * Fh],
                                            in0=rst[:, :Fh], in1=rst[:, :Fh],
                                            op=ALU.mult)
                    stats = smallp.tile([P, 2 * (G - NH)], F32, tag="stats")
                    nc.vector.tensor_reduce(
                        out=stats[:, :2 * NG],
                        in_=pap(zc2, [[HD, 2 * NG], [1, HD]]),
                        axis=AX.X, op=ALU.add)
                    nc.vector.tensor_scalar(out=stats[:, :2 * NG],
                                            in0=stats[:, :2 * NG],
                                            scalar1=1.0 / HD, scalar2=None,
                                            op0=ALU.mult)
                    mu = stats[:, 0:NG]
                    msq = stats[:, NG:2 * NG]
                    var = smallp.tile([P, G - NH], F32, tag="var")
                    nc.vector.tensor_tensor(out=var[:, :NG], in0=mu, in1=mu,
                                            op=ALU.mult)
                    nc.vector.tensor_tensor(out=var[:, :NG], in0=msq,
                                            in1=var[:, :NG], op=ALU.subtract)
                    sd = smallp.tile([P, G - NH], F32, tag="sd")
                    nc.scalar.activation(out=sd[:, :NG], in_=var[:, :NG],
                                         func=ACTF.Sqrt, bias=eps_ap)
                    nc.vector.reciprocal_approx_fast(out=sd[:, :NG],
                                                     in_=sd[:, :NG])
                    mrs = smallp.tile([P, 2 * (G - NH)], BF16, tag="mrs")
                    nc.vector.tensor_copy(out=mrs[:, :NG], in_=mu)
                    nc.vector.tensor_copy(out=mrs[:, NG:2 * NG],
                                          in_=sd[:, :NG])

                    # xhat = (rst - mu) * rstd; out = prelu(xhat*gamma + beta)
                    mub = pap(mrs, [[1, NG], [0, HD]])
                    nc.gpsimd.tensor_tensor(out=rst[:, :Fh], in0=rst[:, :Fh],
                                            in1=mub, op=ALU.subtract)
                    rsb = pap(mrs, [[1, NG], [0, HD]], off=NG)
                    nc.gpsimd.tensor_tensor(out=rst[:, :Fh], in0=rst[:, :Fh],
                                            in1=rsb, op=ALU.mult)
                    gmb = pap(parb, [[0, NG], [1, HD]], off=2 * HD)
                    nc.vector.tensor_tensor(out=rst[:, :Fh], in0=rst[:, :Fh],
                                            in1=gmb, op=ALU.mult)
                    btb = pap(parb, [[0, NG], [1, HD]], off=3 * HD)
                    nc.vector.tensor_tensor(out=rst[:, :Fh], in0=rst[:, :Fh],
                                            in1=btb, op=ALU.add)
                    outf = nodep.tile([P, (G - NH) * HD], F32, tag="outf")
                    nc.scalar.activation(out=outf[:, :Fh], in_=rst[:, :Fh],
                                         func=ACTF.Prelu, alpha=pa_ap)
                    nc.sync.dma_start(out=out_d[:, h0 * HD:h1 * HD],
                                      in_=outf[:, :Fh])

    nc.compile()
    return nc


# ------------------------------------------------------------------- driver

_CACHE = {}


def _get_nc(plan, ncores):
    key = (tuple(int(k) for g0, g1, k in plan["runs"]),
           tuple(g1 - g0 for g0, g1, k in plan["runs"]),
           plan["grid"], ncores)
    if key not in _CACHE:
        _CACHE[key] = _build_nc(plan, ncores)
    return _CACHE[key]


def _make_inmaps(plan, params, ncores):
    (Wk, bk, Wskip, bskip, Wgate, bgate, ln_gamma, ln_beta, prelu_a) = params
    Wk = np.asarray(Wk, np.float32)
    bk = np.asarray(bk, np.float32)
    Wsp = np.asarray(Wskip, np.float32)[:, _PERM]
    bsp = np.asarray(bskip, np.float32)[_PERM]
    wcat = np.zeros((IN_F + 1, P), np.float32)
    wcat[:IN_F, :HD] = Wk
    wcat[IN_F, :HD] = bk
    wcat[:IN_F, HD:] = Wsp
    wcat[IN_F, HD:] = bsp
    wcat = wcat.astype(BF)

    wg = np.asarray(Wgate, np.float32).reshape(3 * HD)
    parb = np.zeros((1, 4 * HD), np.float32)
    parb[0, 0:HD] = (wg[0:HD] + wg[2 * HD:3 * HD])[_PERM]       # on skip
    parb[0, HD:2 * HD] = (wg[HD:2 * HD] - wg[2 * HD:3 * HD])[_PERM]  # on rst
    parb[0, 2 * HD:3 * HD] = np.asarray(ln_gamma, np.float32)[_PERM]
    parb[0, 3 * HD:4 * HD] = np.asarray(ln_beta, np.float32)[_PERM]
    parb = parb.astype(BF)
    parf = np.zeros((1, 4), np.float32)
    parf[0, 0] = np.float32(np.asarray(bgate).reshape(-1)[0])
    parf[0, 1] = np.float32(np.asarray(prelu_a).reshape(-1)[0])
    parf[0, 2] = 1e-5

    in_maps = []
    for c in range(ncores):
        pc = plan["per_core"][c]
        m = dict(featT=plan["featTs"][c], tab=pc["tab"],
                 maskneg=pc["maskneg"], ident=plan["ident"],
                 wcat=wcat, parb=parb, parf=parf)
        in_maps.append(m)
    return in_maps


def run(q_src, v_src, feat, src, dst, Wk, bk, Wskip, bskip, Wgate, bgate,
        ln_gamma, ln_beta, prelu_a, ncores=NCORES, trace=False):
    plan = _plan(q_src, v_src, feat, src, dst, ncores)
    nc = _get_nc(plan, ncores)
    in_maps = _make_inmaps(
        plan, (Wk, bk, Wskip, bskip, Wgate, bgate, ln_gamma, ln_beta, prelu_a),
        ncores)
    res = run_bass_kernel_spmd(nc, in_maps, core_ids=list(range(ncores)),
                               trace=trace)
    n, npc, ngrp = plan["n"], plan["npc"], plan["ngrp"]
    out = np.empty((n, HD), np.float32)
    for c in range(ncores):
        r = np.asarray(res.results[c]["out"])              # [128, ngrp*64]
        r = r.reshape(P, ngrp, D, H).transpose(1, 0, 3, 2)  # -> [g, p, h, d]
        arr = r.reshape(-1, HD)
        out[c * npc + plan["cores"][c]["perm"]] = \
            arr[plan["ndum"]:plan["ndum"] + npc]
    return out, res, plan, in_maps, nc


def kernel(**inputs):
    out, _, _, _, _ = run(**inputs)
    return out
